# revision 3
# baseline (speedup 1.0000x reference)
"""MoE layer (router + 8 experts top-2 + shared expert) on 8 Trainium2 cores.

Strategy (expert-parallel, matching the all-to-all dispatch hint):
  - Host computes router logits/top-2/softmax and gathers each expert's
    tokens (the "all-to-all dispatch" — host-side since kernel() owns the
    full inputs and sharding).
  - Core c holds expert c's W1/W2 and computes
        y_c = relu(x_gathered @ W1_c + b1_c) @ W2_c
    for its (padded-to-capacity) token set, in transposed layout so both
    matmul stationary operands come straight from the natural weight layout.
  - The always-on shared expert is d_ff-sliced 8 ways: core c computes
    partial_c = relu(x_all @ Ws1[:, c*512:(c+1)*512] + bs1[slice]) @ Ws2[slice]
    over all tokens; partials are summed on host.
  - Host applies gate weights, b2/bs2 biases, and scatter-adds expert
    outputs back to token order.

All matmuls run as float32r (full-rate fp32 path on the PE array) with
fp32 PSUM accumulation.
"""

import os
import sys

import numpy as np

for _p in ("/opt/trn_rl_repo", os.path.expanduser("~/.axon_site/_ro/trn_rl_repo")):
    if os.path.isdir(_p) and _p not in sys.path:
        sys.path.append(_p)

import concourse.bass as bass  # noqa: E402
import concourse.tile as tile  # noqa: E402
from concourse import bacc, mybir  # noqa: E402
from concourse.bass import ds, ts  # noqa: E402
from concourse.bass_utils import run_bass_kernel_spmd  # noqa: E402

D_MODEL, D_FF, N_EXP, TOP_K = 1024, 4096, 8, 2
P = 128
KD = D_MODEL // P        # 8 partition-tiles over d_model
MF = D_FF // P           # 32 partition-tiles over d_ff
FF_SH = D_FF // N_EXP    # 512: shared-expert d_ff slice per core
MS = FF_SH // P          # 4 partition-tiles over the shared slice
T_TOTAL = 4096
TT = 1024                # shared-expert token tile

F32 = mybir.dt.float32
F32R = mybir.dt.float32r
RELU = mybir.ActivationFunctionType.Relu
ADD = mybir.AluOpType.add


def _col_slices(n, step=512):
    return [(off, min(step, n - off)) for off in range(0, n, step)]


def build_program(C, n_iter=1):
    """SPMD program for capacity-C tokens per expert. n_iter repeats the
    whole compute (for timing); outputs are simply overwritten."""
    nc = bacc.Bacc(None, target_bir_lowering=False, debug=False)

    xg = nc.declare_dram_parameter("xg", [P, KD, C], F32R, isOutput=False)
    xt = nc.declare_dram_parameter("xt", [P, KD, T_TOTAL], F32R, isOutput=False)
    w1 = nc.declare_dram_parameter("w1", [P, KD, D_FF], F32R, isOutput=False)
    w2 = nc.declare_dram_parameter("w2", [P, MF, D_MODEL], F32R, isOutput=False)
    b1t = nc.declare_dram_parameter("b1t", [P, MF], F32, isOutput=False)
    ws1 = nc.declare_dram_parameter("ws1", [P, KD, FF_SH], F32R, isOutput=False)
    ws2 = nc.declare_dram_parameter("ws2", [P, MS, D_MODEL], F32R, isOutput=False)
    bs1t = nc.declare_dram_parameter("bs1t", [P, MS], F32, isOutput=False)
    yt = nc.declare_dram_parameter("yt", [P, KD, C], F32, isOutput=True)
    st = nc.declare_dram_parameter("st", [P, KD, T_TOTAL], F32, isOutput=True)

    ncs = _col_slices(C)

    with tile.TileContext(nc) as tc:
        for _ in range(n_iter):
            # ---------------- Phase 1: this core's expert on gathered tokens
            with (
                tc.tile_pool(name="const1", bufs=1) as const1,
                tc.tile_pool(name="xg_p", bufs=1) as xg_p,
                tc.tile_pool(name="y_p", bufs=1) as y_p,
                tc.tile_pool(name="w1_p", bufs=2) as w1_p,
                tc.tile_pool(name="w2_p", bufs=2) as w2_p,
                tc.tile_pool(name="h_p", bufs=2) as h_p,
                tc.tile_pool(name="ph", bufs=3, space="PSUM") as ph,
                tc.tile_pool(name="py", bufs=3, space="PSUM") as py,
            ):
                b1_sb = const1.tile([P, MF], F32)
                nc.sync.dma_start(out=b1_sb[:], in_=b1t[:])
                xg_sb = xg_p.tile([P, KD, C], F32R)
                nc.sync.dma_start(out=xg_sb[:], in_=xg[:])
                y_sb = y_p.tile([P, KD, C], F32)

                for mg in range(MF // 4):  # 8 groups of 4 ff-tiles (512 ff)
                    w1_sb = w1_p.tile([P, KD, 512], F32R, tag="w1")
                    nc.sync.dma_start(out=w1_sb[:], in_=w1[:, :, ds(mg * 512, 512)])
                    w2_sb = w2_p.tile([P, 4, D_MODEL], F32R, tag="w2")
                    nc.sync.dma_start(out=w2_sb[:], in_=w2[:, ds(mg * 4, 4), :])
                    h_sb = h_p.tile([P, 4, C], F32R, tag="h")

                    for m4 in range(4):
                        m = mg * 4 + m4
                        psums = [ph.tile([P, ln], F32, tag="ph", name=f"ph_{i}") for i, (_, ln) in enumerate(ncs)]
                        for k in range(KD):
                            lhsT = w1_sb[:, k, ts(m4, P)]
                            for i, (off, ln) in enumerate(ncs):
                                nc.tensor.matmul(
                                    psums[i][:],
                                    lhsT,
                                    xg_sb[:, k, ds(off, ln)],
                                    start=(k == 0),
                                    stop=(k == KD - 1),
                                )
                        for i, (off, ln) in enumerate(ncs):
                            nc.scalar.activation(
                                out=h_sb[:, m4, ds(off, ln)],
                                in_=psums[i][:],
                                func=RELU,
                                bias=b1_sb[:, m : m + 1],
                            )

                    for j in range(KD):
                        ypsums = [py.tile([P, ln], F32, tag="py", name=f"py_{i}") for i, (_, ln) in enumerate(ncs)]
                        for m4 in range(4):
                            lhsT = w2_sb[:, m4, ts(j, P)]
                            for i, (off, ln) in enumerate(ncs):
                                nc.tensor.matmul(
                                    ypsums[i][:],
                                    lhsT,
                                    h_sb[:, m4, ds(off, ln)],
                                    start=(m4 == 0),
                                    stop=(m4 == 3),
                                )
                        for i, (off, ln) in enumerate(ncs):
                            if mg == 0:
                                nc.scalar.copy(
                                    out=y_sb[:, j, ds(off, ln)], in_=ypsums[i][:]
                                )
                            else:
                                nc.vector.tensor_tensor(
                                    out=y_sb[:, j, ds(off, ln)],
                                    in0=y_sb[:, j, ds(off, ln)],
                                    in1=ypsums[i][:],
                                    op=ADD,
                                )
                nc.sync.dma_start(out=yt[:], in_=y_sb[:])

            # ---------------- Phase 2: shared expert, d_ff slice, all tokens
            with (
                tc.tile_pool(name="const2", bufs=1) as const2,
                tc.tile_pool(name="ws_p", bufs=1) as ws_p,
                tc.tile_pool(name="xt_p", bufs=2) as xt_p,
                tc.tile_pool(name="hs_p", bufs=2) as hs_p,
                tc.tile_pool(name="so_p", bufs=2) as so_p,
                tc.tile_pool(name="ph2", bufs=3, space="PSUM") as ph2,
                tc.tile_pool(name="py2", bufs=3, space="PSUM") as py2,
            ):
                bs1_sb = const2.tile([P, MS], F32)
                nc.sync.dma_start(out=bs1_sb[:], in_=bs1t[:])
                ws1_sb = ws_p.tile([P, KD, FF_SH], F32R, tag="ws1")
                nc.sync.dma_start(out=ws1_sb[:], in_=ws1[:])
                ws2_sb = ws_p.tile([P, MS, D_MODEL], F32R, tag="ws2")
                nc.sync.dma_start(out=ws2_sb[:], in_=ws2[:])

                for tt in range(T_TOTAL // TT):
                    xt_sb = xt_p.tile([P, KD, TT], F32R, tag="xt")
                    nc.sync.dma_start(out=xt_sb[:], in_=xt[:, :, ds(tt * TT, TT)])
                    hs_sb = hs_p.tile([P, MS, TT], F32R, tag="hs")
                    for m in range(MS):
                        psums = [ph2.tile([P, 512], F32, tag="ph2", name=f"ph2_{n}") for n in range(TT // 512)]
                        for k in range(KD):
                            lhsT = ws1_sb[:, k, ts(m, P)]
                            for n in range(TT // 512):
                                nc.tensor.matmul(
                                    psums[n][:],
                                    lhsT,
                                    xt_sb[:, k, ds(n * 512, 512)],
                                    start=(k == 0),
                                    stop=(k == KD - 1),
                                )
                        for n in range(TT // 512):
                            nc.scalar.activation(
                                out=hs_sb[:, m, ds(n * 512, 512)],
                                in_=psums[n][:],
                                func=RELU,
                                bias=bs1_sb[:, m : m + 1],
                            )
                    s_sb = so_p.tile([P, KD, TT], F32, tag="so")
                    for j in range(KD):
                        ypsums = [py2.tile([P, 512], F32, tag="py2", name=f"py2_{n}") for n in range(TT // 512)]
                        for m in range(MS):
                            lhsT = ws2_sb[:, m, ts(j, P)]
                            for n in range(TT // 512):
                                nc.tensor.matmul(
                                    ypsums[n][:],
                                    lhsT,
                                    hs_sb[:, m, ds(n * 512, 512)],
                                    start=(m == 0),
                                    stop=(m == MS - 1),
                                )
                        for n in range(TT // 512):
                            nc.scalar.copy(
                                out=s_sb[:, j, ds(n * 512, 512)], in_=ypsums[n][:]
                            )
                    nc.sync.dma_start(out=st[:, :, ds(tt * TT, TT)], in_=s_sb[:])

    nc.compile()
    return nc


def _to_tiles(a2d):
    """[R, N] with R = r_tiles*128 -> [128, r_tiles, N] so element
    [p, r, n] = a2d[r*128 + p, n]; contiguous for a single straight DMA."""
    R, N = a2d.shape
    return np.ascontiguousarray(
        a2d.reshape(R // P, P, N).transpose(1, 0, 2)
    )


def _from_tiles(a3d):
    """Inverse of _to_tiles: [128, r_tiles, N] -> [r_tiles*128, N]."""
    p, r, n = a3d.shape
    return a3d.transpose(1, 0, 2).reshape(r * p, n)


def _route(xf, Wg):
    """Replicates TopKRouter eval: top-2 by logit, softmax over the two."""
    logits = xf @ Wg
    top_idx = np.argsort(-logits, axis=1, kind="stable")[:, :TOP_K]
    top_vals = np.take_along_axis(logits, top_idx, axis=1)
    e = np.exp(top_vals - top_vals.max(axis=1, keepdims=True))
    top_w = (e / e.sum(axis=1, keepdims=True)).astype(np.float32)
    return top_idx, top_w


_PROG_CACHE = {}


def _get_program(C):
    if C not in _PROG_CACHE:
        _PROG_CACHE[C] = build_program(C)
    return _PROG_CACHE[C]


def make_in_maps(x, Wg, W1, b1, W2, b2, Ws1, bs1, Ws2, bs2):
    """Host-side routing + sharding. Returns (in_maps, C, idx_e, gate_e, xf)."""
    B, S, D = x.shape
    T = B * S
    xf = np.ascontiguousarray(np.asarray(x, np.float32).reshape(T, D))
    top_idx, top_w = _route(xf, np.asarray(Wg, np.float32))

    idx_e, gate_e = [], []
    for ex in range(N_EXP):
        rows, slot = np.nonzero(top_idx == ex)
        idx_e.append(rows)
        gate_e.append(top_w[rows, slot])
    counts = [len(i) for i in idx_e]
    C = max(P, -(-max(counts) // P) * P)

    xt_tiled = _to_tiles(xf.T)  # [128, 8, 4096]
    in_maps = []
    for ex in range(N_EXP):
        xg = np.zeros((C, D_MODEL), np.float32)
        xg[: counts[ex]] = xf[idx_e[ex]]
        sl = slice(ex * FF_SH, (ex + 1) * FF_SH)
        in_maps.append(
            {
                "xg": _to_tiles(np.ascontiguousarray(xg.T)),
                "xt": xt_tiled,
                "w1": _to_tiles(np.asarray(W1[ex], np.float32)),
                "w2": _to_tiles(np.asarray(W2[ex], np.float32)),
                "b1t": np.ascontiguousarray(
                    np.asarray(b1[ex], np.float32).reshape(MF, P).T
                ),
                "ws1": _to_tiles(np.asarray(Ws1[:, sl], np.float32)),
                "ws2": _to_tiles(np.asarray(Ws2[sl, :], np.float32)),
                "bs1t": np.ascontiguousarray(
                    np.asarray(bs1[sl], np.float32).reshape(MS, P).T
                ),
            }
        )
    return in_maps, C, idx_e, gate_e, xf


def assemble_output(results, shape, C, idx_e, gate_e, b2, bs2):
    B, S, D = shape
    T = B * S
    out = np.zeros((T, D), np.float32)
    for ex in range(N_EXP):
        out += _from_tiles(results[ex]["st"]).T  # shared partials
    out += np.asarray(bs2, np.float32)[None, :]
    b2 = np.asarray(b2, np.float32)
    for ex in range(N_EXP):
        y = _from_tiles(results[ex]["yt"]).T[: len(idx_e[ex])]
        out[idx_e[ex]] += gate_e[ex][:, None] * (y + b2[ex][None, :])
    return out.reshape(B, S, D)


def kernel(x, Wg, W1, b1, W2, b2, Ws1, bs1, Ws2, bs2):
    in_maps, C, idx_e, gate_e, _ = make_in_maps(
        x, Wg, W1, b1, W2, b2, Ws1, bs1, Ws2, bs2
    )
    nc = _get_program(C)
    res = run_bass_kernel_spmd(nc, in_maps, list(range(N_EXP)))
    return assemble_output(
        res.results, x.shape, C, idx_e, gate_e, b2, bs2
    ).astype(np.float32)
